# revision 4
# baseline (speedup 1.0000x reference)
"""AWQ 4-bit quantized linear (group size 128) on 8 Trainium2 NeuronCores.

Column-parallel: each core owns OUT/8 = 1376 output columns. The host does
layout-only prep (slicing, int4->uint8 nibble widening with the AWQ column
permutation, transposes); all arithmetic — zero-point subtract, scale
multiply, matmul, bias — runs on device.

Per-core device pipeline, for each 128-row block of output columns (o-tile):
  1. DMA the packed-weight rows (uint8 nibbles, o on partitions).
  2. DVE tensor_scalar dual-op dequant: w = nib * s[o] - (z*s)[o]  (fp16 out).
     Scales/zeros vary per (o, group); with o on partitions they are
     per-partition scalars, which tensor_scalar supports natively.
  3. DMA xbar transpose each [o=128, k=128] fp16 tile to [k=128, o=128]
     (matmul needs the contraction dim on partitions).
  4. PE matmul accumulation over the 32 k-groups into PSUM:
     outT[o, m] += w_g[k, o].T @ xT_g[k, m].
  5. ACT evacuation: out = Identity(psum + bias[o]) -> fp16, DMA to DRAM.
"""

import os
import sys

import numpy as np

if "/opt/trn_rl_repo" not in sys.path:
    sys.path.insert(0, "/opt/trn_rl_repo")

M, IN, OUT = 1024, 4096, 11008
N_CORES = 8
OC = OUT // N_CORES  # 1376 output columns per core
GS = 128  # quantization group size (== matmul k-tile)
G = IN // GS  # 32 groups
PACK = 8  # int4 values per int32 word
# reference unpacks nibble k to logical column AWQ_REVERSE_ORDER.index(k);
# equivalently logical column j within a word uses shift 4*REV[j]:
REV = np.array([0, 4, 1, 5, 2, 6, 3, 7], dtype=np.uint32)

MM_N = 512  # moving-operand free size per matmul (one PSUM bank of fp32)

_CACHE = {}


def _unpack_int4(q: np.ndarray) -> np.ndarray:
    """[rows, cols//8] int32 -> [rows, cols] uint8 in 0..15 (AWQ order)."""
    qu = q.view(np.uint32)
    nib = (qu[:, :, None] >> (REV * 4)[None, None, :]) & 0xF
    return nib.reshape(q.shape[0], -1).astype(np.uint8)


def _build(m, k, oc, n_cores):
    import concourse.bacc as bacc
    import concourse.tile as tile
    from concourse import mybir

    F16 = mybir.dt.float16
    F32 = mybir.dt.float32
    U8 = mybir.dt.uint8

    g = k // GS
    n_otiles = (oc + 127) // 128
    n_mch = (m + MM_N - 1) // MM_N

    nc = bacc.Bacc("TRN2", target_bir_lowering=False, debug=False)
    xT = nc.dram_tensor("xT", [k, m], F16, kind="ExternalInput").ap()
    qw8T = nc.dram_tensor("qw8T", [oc, k], U8, kind="ExternalInput").ap()
    sT = nc.dram_tensor("sT", [oc, g], F32, kind="ExternalInput").ap()
    z8T = nc.dram_tensor("z8T", [oc, g], U8, kind="ExternalInput").ap()
    biasT = nc.dram_tensor("biasT", [oc, 1], F16, kind="ExternalInput").ap()
    outT = nc.dram_tensor("outT", [oc, m], F16, kind="ExternalOutput").ap()

    # quarter-otile granularity: groups processed in blocks of QG so the
    # first matmuls are unblocked early and otile handoff stays smooth
    QG = min(8, g)
    n_q = g // QG
    # dequant engine per group index within an otile (DVE fastest, ACT, GPS)
    deq_engine = ([0] * 17 + [1] * 9 + [2] * 6) if g == 32 else [0] * g

    with tile.TileContext(nc) as tc:
        with (
            tc.tile_pool(name="x", bufs=1) as xpool,
            tc.tile_pool(name="consts", bufs=3) as cpool,
            tc.tile_pool(name="qw", bufs=3) as qwpool,
            tc.tile_pool(name="wd", bufs=3 * n_q) as wdpool,
            tc.tile_pool(name="w", bufs=3 * n_q) as wpool,
            tc.tile_pool(name="ps", bufs=8, space="PSUM") as pspool,
            tc.tile_pool(name="o", bufs=4) as opool,
        ):
            # resident transposed activations: [128, g, m]
            xT_sb = xpool.tile([128, g, m], F16)
            xT_r = xT.rearrange("(gg p) mm -> p gg mm", p=128)

            def load_x(g0, g1):
                for gg in range(g0, g1, 2):
                    gh = min(g1, gg + 2)
                    nc.gpsimd.dma_start(xT_sb[:, gg:gh], xT_r[:, gg:gh])

            # first quarter's x right away; the rest after otile-0 prep is queued
            load_x(0, QG)

            def prep_otile(ot):
                o0 = ot * 128
                ob = min(128, oc - o0)

                s_t = cpool.tile([ob, g], F32, tag="sT")
                z8_t = cpool.tile([ob, g], U8, tag="z8")
                zf_t = cpool.tile([ob, g], F32, tag="zf")
                zs_t = cpool.tile([ob, g], F32, tag="zs")
                nzs_t = cpool.tile([ob, g], F32, tag="nzs")
                b_t = cpool.tile([ob, 1], F16, tag="b")
                nc.sync.dma_start(s_t[:], sT[o0 : o0 + ob])
                nc.sync.dma_start(z8_t[:], z8T[o0 : o0 + ob])
                nc.sync.dma_start(b_t[:], biasT[o0 : o0 + ob])
                nc.vector.tensor_copy(zf_t[:], z8_t[:])
                nc.vector.tensor_tensor(
                    zs_t[:], zf_t[:], s_t[:], mybir.AluOpType.mult
                )
                nc.vector.tensor_scalar(
                    nzs_t[:], zs_t[:], -1.0, None, mybir.AluOpType.mult
                )

                qw_t = qwpool.tile([ob, k], U8)
                nc.gpsimd.dma_start(qw_t[:], qw8T[o0 : o0 + ob])

                # dequant + transpose per quarter (QG groups each)
                w3_qs = []
                for q in range(n_q):
                    wd_t = wdpool.tile([ob, QG * GS], F16, tag="wd")
                    for j in range(QG):
                        gi = q * QG + j
                        ksl_q = slice(j * GS, (j + 1) * GS)
                        ksl = slice(gi * GS, (gi + 1) * GS)
                        eng = (nc.vector, nc.scalar, nc.gpsimd)[deq_engine[gi]]
                        if eng is nc.scalar:
                            nc.scalar.activation(
                                wd_t[:, ksl_q],
                                qw_t[:, ksl],
                                mybir.ActivationFunctionType.Identity,
                                bias=nzs_t[:, gi : gi + 1],
                                scale=s_t[:, gi : gi + 1],
                            )
                        else:
                            eng.tensor_scalar(
                                wd_t[:, ksl_q],
                                qw_t[:, ksl],
                                s_t[:, gi : gi + 1],
                                zs_t[:, gi : gi + 1],
                                mybir.AluOpType.mult,
                                mybir.AluOpType.subtract,
                            )
                    # xbar transpose [ob, QG*GS] -> [128, QG, ob]
                    w3_t = wpool.tile([128, QG, ob], F16, tag="w")
                    nc.sync.dma_start_transpose(w3_t[:], wd_t[:])
                    w3_qs.append(w3_t)
                return o0, ob, b_t, w3_qs

            def mm_otile(ot, prep):
                o0, ob, b_t, w3_qs = prep
                for mc in range(n_mch):
                    msl = slice(mc * MM_N, min(m, (mc + 1) * MM_N))
                    mn = msl.stop - msl.start
                    ps = pspool.tile([ob, MM_N], F32)
                    for gi in range(g):
                        nc.tensor.matmul(
                            ps[:, :mn],
                            w3_qs[gi // QG][:, gi % QG, :],
                            xT_sb[:, gi, msl],
                            start=(gi == 0),
                            stop=(gi == g - 1),
                        )
                    o_t = opool.tile([ob, MM_N], F16)
                    nc.scalar.activation(
                        o_t[:, :mn],
                        ps[:, :mn],
                        mybir.ActivationFunctionType.Identity,
                        bias=b_t[:],
                        scale=1.0,
                    )
                    nc.sync.dma_start(outT[o0 : o0 + ob, msl], o_t[:, :mn])

            prep = prep_otile(0)
            load_x(QG, g)
            for ot in range(n_otiles):
                nxt = prep_otile(ot + 1) if ot + 1 < n_otiles else None
                mm_otile(ot, prep)
                prep = nxt

    nc.compile()
    return nc


def _get_nc(m=M, k=IN, oc=OC, n_cores=N_CORES):
    key = (m, k, oc, n_cores)
    if key not in _CACHE:
        _CACHE[key] = _build(*key)
    return _CACHE[key]


def _make_in_maps(x, qweight, qzeros, scales, bias, n_cores=N_CORES):
    iw8 = _unpack_int4(qweight)  # [IN, OUT] uint8
    iz8 = _unpack_int4(qzeros)  # [G, OUT] uint8
    xT = np.ascontiguousarray(x.T)  # [IN, M] fp16
    oc = qweight.shape[1] * PACK // n_cores
    in_maps = []
    for c in range(n_cores):
        sl = slice(c * oc, (c + 1) * oc)
        in_maps.append(
            {
                "xT": xT,
                "qw8T": np.ascontiguousarray(iw8[:, sl].T),
                "sT": np.ascontiguousarray(scales[:, sl].T.astype(np.float32)),
                "z8T": np.ascontiguousarray(iz8[:, sl].T),
                "biasT": np.ascontiguousarray(bias[sl].reshape(-1, 1)),
            }
        )
    return in_maps


LAST_EXEC_NS = None


def kernel(x, qweight, qzeros, scales, bias):
    global LAST_EXEC_NS
    from concourse.bass_utils import run_bass_kernel_spmd

    x = np.asarray(x)
    qweight = np.asarray(qweight)
    qzeros = np.asarray(qzeros)
    scales = np.asarray(scales)
    bias = np.asarray(bias)

    nc = _get_nc()
    in_maps = _make_in_maps(x, qweight, qzeros, scales, bias)

    kwargs = {}
    if os.environ.get("AWQ_PROFILE"):
        _enable_profiling()
        kwargs = dict(trace=True, tmpdir=os.environ.get("AWQ_TRACE_DIR") or None)
    res = run_bass_kernel_spmd(nc, in_maps, list(range(N_CORES)), **kwargs)
    LAST_EXEC_NS = res.exec_time_ns

    outT = np.concatenate([res.results[c]["outT"] for c in range(N_CORES)], axis=0)
    return np.ascontiguousarray(outT.T)


def _enable_profiling():
    """Register the NTFF profile hook missing from this image's antenv."""
    import types

    if "antenv.axon_hooks" not in sys.modules:
        import antenv

        mod = types.ModuleType("antenv.axon_hooks")
        mod._hook = None
        mod.set_axon_ntff_profile_hook = lambda h: setattr(mod, "_hook", h)
        mod.get_axon_ntff_profile_hook = lambda: mod._hook
        sys.modules["antenv.axon_hooks"] = mod
        antenv.axon_hooks = mod
        try:
            from trn_agent_boot.trn_boot import _ntff_profile_via_ctypes

            mod.set_axon_ntff_profile_hook(
                _ntff_profile_via_ctypes("/opt/axon/libaxon_pjrt.so")
            )
        except Exception:
            pass
    import concourse.bass_utils as _bu

    _bu.upload_artifacts = lambda tmpdir: "local://skipped"


# revision 7
# speedup vs baseline: 1.0909x; 1.0909x over previous
"""AWQ 4-bit quantized linear (group size 128) on 8 Trainium2 NeuronCores.

Column-parallel: each core owns OUT/8 = 1376 output columns. The host does
layout-only prep (slicing, int4->uint8 nibble widening with the AWQ column
permutation, transposes); all arithmetic — zero-point subtract, scale
multiply, matmul, bias — runs on device.

Per-core device pipeline, for each 128-row block of output columns (o-tile):
  1. DMA the packed-weight rows (uint8 nibbles, o on partitions).
  2. DVE tensor_scalar dual-op dequant: w = nib * s[o] - (z*s)[o]  (fp16 out).
     Scales/zeros vary per (o, group); with o on partitions they are
     per-partition scalars, which tensor_scalar supports natively.
  3. DMA xbar transpose each [o=128, k=128] fp16 tile to [k=128, o=128]
     (matmul needs the contraction dim on partitions).
  4. PE matmul accumulation over the 32 k-groups into PSUM:
     outT[o, m] += w_g[k, o].T @ xT_g[k, m].
  5. ACT evacuation: out = Identity(psum + bias[o]) -> fp16, DMA to DRAM.
"""

import os
import sys

import numpy as np

if "/opt/trn_rl_repo" not in sys.path:
    sys.path.insert(0, "/opt/trn_rl_repo")

M, IN, OUT = 1024, 4096, 11008
N_CORES = 8
OC = OUT // N_CORES  # 1376 output columns per core
GS = 128  # quantization group size (== matmul k-tile)
G = IN // GS  # 32 groups
PACK = 8  # int4 values per int32 word
# reference unpacks nibble k to logical column AWQ_REVERSE_ORDER.index(k);
# equivalently logical column j within a word uses shift 4*REV[j]:
REV = np.array([0, 4, 1, 5, 2, 6, 3, 7], dtype=np.uint32)

MM_N = 512  # moving-operand free size per matmul (one PSUM bank of fp32)

_CACHE = {}


def _unpack_int4(q: np.ndarray) -> np.ndarray:
    """[rows, cols//8] int32 -> [rows, cols] uint8 in 0..15 (AWQ order)."""
    qu = q.view(np.uint32)
    nib = (qu[:, :, None] >> (REV * 4)[None, None, :]) & 0xF
    return nib.reshape(q.shape[0], -1).astype(np.uint8)


def _build(m, k, oc, n_cores):
    import concourse.bacc as bacc
    import concourse.tile as tile
    from concourse import mybir

    F16 = mybir.dt.float16
    F32 = mybir.dt.float32
    U8 = mybir.dt.uint8

    g = k // GS
    n_otiles = (oc + 127) // 128
    n_mch = (m + MM_N - 1) // MM_N

    nc = bacc.Bacc("TRN2", target_bir_lowering=False, debug=False)
    xT = nc.dram_tensor("xT", [k, m], F16, kind="ExternalInput").ap()
    qw8T = nc.dram_tensor("qw8T", [oc, k], U8, kind="ExternalInput").ap()
    sT = nc.dram_tensor("sT", [oc, g], F32, kind="ExternalInput").ap()
    z8T = nc.dram_tensor("z8T", [oc, g], U8, kind="ExternalInput").ap()
    biasT = nc.dram_tensor("biasT", [oc, 1], F16, kind="ExternalInput").ap()
    outT = nc.dram_tensor("outT", [oc, m], F16, kind="ExternalOutput").ap()

    # quarter-otile granularity: groups processed in blocks of QG so the
    # first matmuls are unblocked early and otile handoff stays smooth
    QG = min(8, g)
    n_q = g // QG
    # dequant engine per group position within a quarter: 5 DVE, 3 ACT
    # (GPSIMD intentionally unused: it port-muxes with DVE and is ~7x slower)
    deq_pattern = [0, 1, 0, 0, 1, 0, 0, 1]

    with tile.TileContext(nc) as tc:
        with (
            tc.tile_pool(name="x", bufs=1) as xpool,
            tc.tile_pool(name="consts", bufs=3) as cpool,
            tc.tile_pool(name="qw", bufs=3) as qwpool,
            tc.tile_pool(name="wd", bufs=3 * n_q) as wdpool,
            tc.tile_pool(name="w", bufs=3 * n_q) as wpool,
            tc.tile_pool(name="ps", bufs=8, space="PSUM") as pspool,
            tc.tile_pool(name="o", bufs=4) as opool,
        ):
            # resident transposed activations: [128, g, m]
            xT_sb = xpool.tile([128, g, m], F16)
            xT_r = xT.rearrange("(gg p) mm -> p gg mm", p=128)

            def load_x(g0, g1):
                for gg in range(g0, g1, 2):
                    gh = min(g1, gg + 2)
                    nc.gpsimd.dma_start(xT_sb[:, gg:gh], xT_r[:, gg:gh])

            # first quarter's x right away; the rest after otile-0 prep is queued
            load_x(0, QG)

            def prep_otile(ot):
                o0 = ot * 128
                ob = min(128, oc - o0)

                s_t = cpool.tile([ob, g], F32, tag="sT")
                z8_t = cpool.tile([ob, g], U8, tag="z8")
                zf_t = cpool.tile([ob, g], F32, tag="zf")
                zs_t = cpool.tile([ob, g], F32, tag="zs")
                nzs_t = cpool.tile([ob, g], F32, tag="nzs")
                b_t = cpool.tile([ob, 1], F16, tag="b")
                nc.sync.dma_start(s_t[:], sT[o0 : o0 + ob])
                nc.sync.dma_start(z8_t[:], z8T[o0 : o0 + ob])
                nc.sync.dma_start(b_t[:], biasT[o0 : o0 + ob])
                nc.vector.tensor_copy(zf_t[:], z8_t[:])
                nc.vector.tensor_tensor(
                    zs_t[:], zf_t[:], s_t[:], mybir.AluOpType.mult
                )
                nc.vector.tensor_scalar(
                    nzs_t[:], zs_t[:], -1.0, None, mybir.AluOpType.mult
                )

                qw_t = qwpool.tile([ob, k], U8)
                for q in range(n_q):
                    qsl = slice(q * QG * GS, (q + 1) * QG * GS)
                    nc.gpsimd.dma_start(qw_t[:, qsl], qw8T[o0 : o0 + ob, qsl])

                # dequant + transpose per quarter (QG groups each)
                w3_qs = []
                for q in range(n_q):
                    wd_t = wdpool.tile([ob, QG * GS], F16, tag="wd")
                    for j in range(QG):
                        gi = q * QG + j
                        ksl_q = slice(j * GS, (j + 1) * GS)
                        ksl = slice(gi * GS, (gi + 1) * GS)
                        if deq_pattern[j % len(deq_pattern)]:
                            nc.scalar.activation(
                                wd_t[:, ksl_q],
                                qw_t[:, ksl],
                                mybir.ActivationFunctionType.Identity,
                                bias=nzs_t[:, gi : gi + 1],
                                scale=s_t[:, gi : gi + 1],
                            )
                        else:
                            nc.vector.tensor_scalar(
                                wd_t[:, ksl_q],
                                qw_t[:, ksl],
                                s_t[:, gi : gi + 1],
                                zs_t[:, gi : gi + 1],
                                mybir.AluOpType.mult,
                                mybir.AluOpType.subtract,
                            )
                    # xbar transpose [ob, QG*GS] -> [128, QG, ob]
                    w3_t = wpool.tile([128, QG, ob], F16, tag="w")
                    nc.sync.dma_start_transpose(w3_t[:], wd_t[:])
                    w3_qs.append(w3_t)
                return o0, ob, b_t, w3_qs

            def mm_otile(ot, prep):
                o0, ob, b_t, w3_qs = prep
                for mc in range(n_mch):
                    msl = slice(mc * MM_N, min(m, (mc + 1) * MM_N))
                    mn = msl.stop - msl.start
                    ps = pspool.tile([ob, MM_N], F32)
                    for gi in range(g):
                        nc.tensor.matmul(
                            ps[:, :mn],
                            w3_qs[gi // QG][:, gi % QG, :],
                            xT_sb[:, gi, msl],
                            start=(gi == 0),
                            stop=(gi == g - 1),
                        )
                    o_t = opool.tile([ob, MM_N], F16)
                    nc.scalar.activation(
                        o_t[:, :mn],
                        ps[:, :mn],
                        mybir.ActivationFunctionType.Identity,
                        bias=b_t[:],
                        scale=1.0,
                    )
                    nc.sync.dma_start(outT[o0 : o0 + ob, msl], o_t[:, :mn])

            prep = prep_otile(0)
            load_x(QG, g)
            for ot in range(n_otiles):
                nxt = prep_otile(ot + 1) if ot + 1 < n_otiles else None
                mm_otile(ot, prep)
                prep = nxt

    nc.compile()
    return nc


def _get_nc(m=M, k=IN, oc=OC, n_cores=N_CORES):
    key = (m, k, oc, n_cores)
    if key not in _CACHE:
        _CACHE[key] = _build(*key)
    return _CACHE[key]


def _make_in_maps(x, qweight, qzeros, scales, bias, n_cores=N_CORES):
    iw8 = _unpack_int4(qweight)  # [IN, OUT] uint8
    iz8 = _unpack_int4(qzeros)  # [G, OUT] uint8
    xT = np.ascontiguousarray(x.T)  # [IN, M] fp16
    oc = qweight.shape[1] * PACK // n_cores
    in_maps = []
    for c in range(n_cores):
        sl = slice(c * oc, (c + 1) * oc)
        in_maps.append(
            {
                "xT": xT,
                "qw8T": np.ascontiguousarray(iw8[:, sl].T),
                "sT": np.ascontiguousarray(scales[:, sl].T.astype(np.float32)),
                "z8T": np.ascontiguousarray(iz8[:, sl].T),
                "biasT": np.ascontiguousarray(bias[sl].reshape(-1, 1)),
            }
        )
    return in_maps


LAST_EXEC_NS = None


def kernel(x, qweight, qzeros, scales, bias):
    global LAST_EXEC_NS
    from concourse.bass_utils import run_bass_kernel_spmd

    x = np.asarray(x)
    qweight = np.asarray(qweight)
    qzeros = np.asarray(qzeros)
    scales = np.asarray(scales)
    bias = np.asarray(bias)

    nc = _get_nc()
    in_maps = _make_in_maps(x, qweight, qzeros, scales, bias)

    kwargs = {}
    if os.environ.get("AWQ_PROFILE"):
        _enable_profiling()
        kwargs = dict(trace=True, tmpdir=os.environ.get("AWQ_TRACE_DIR") or None)
    res = run_bass_kernel_spmd(nc, in_maps, list(range(N_CORES)), **kwargs)
    LAST_EXEC_NS = res.exec_time_ns

    outT = np.concatenate([res.results[c]["outT"] for c in range(N_CORES)], axis=0)
    return np.ascontiguousarray(outT.T)


def _enable_profiling():
    """Register the NTFF profile hook missing from this image's antenv."""
    import types

    if "antenv.axon_hooks" not in sys.modules:
        import antenv

        mod = types.ModuleType("antenv.axon_hooks")
        mod._hook = None
        mod.set_axon_ntff_profile_hook = lambda h: setattr(mod, "_hook", h)
        mod.get_axon_ntff_profile_hook = lambda: mod._hook
        sys.modules["antenv.axon_hooks"] = mod
        antenv.axon_hooks = mod
        try:
            from trn_agent_boot.trn_boot import _ntff_profile_via_ctypes

            mod.set_axon_ntff_profile_hook(
                _ntff_profile_via_ctypes("/opt/axon/libaxon_pjrt.so")
            )
        except Exception:
            pass
    import concourse.bass_utils as _bu

    _bu.upload_artifacts = lambda tmpdir: "local://skipped"


# revision 10
# speedup vs baseline: 1.1829x; 1.0843x over previous
"""AWQ 4-bit quantized linear (group size 128) on 8 Trainium2 NeuronCores.

Column-parallel: each core owns OUT/8 = 1376 output columns. The host does
layout-only prep (slicing, int4->uint8 nibble widening with the AWQ column
permutation, transposes); all arithmetic — zero-point subtract, scale
multiply, matmul, bias — runs on device.

Per-core device pipeline, for each 128-row block of output columns (o-tile):
  1. DMA the packed-weight rows (uint8 nibbles, o on partitions).
  2. DVE tensor_scalar dual-op dequant: w = nib * s[o] - (z*s)[o]  (fp16 out).
     Scales/zeros vary per (o, group); with o on partitions they are
     per-partition scalars, which tensor_scalar supports natively.
  3. DMA xbar transpose each [o=128, k=128] fp16 tile to [k=128, o=128]
     (matmul needs the contraction dim on partitions).
  4. PE matmul accumulation over the 32 k-groups into PSUM:
     outT[o, m] += w_g[k, o].T @ xT_g[k, m].
  5. ACT evacuation: out = Identity(psum + bias[o]) -> fp16, DMA to DRAM.
"""

import os
import sys

import numpy as np

if "/opt/trn_rl_repo" not in sys.path:
    sys.path.insert(0, "/opt/trn_rl_repo")

M, IN, OUT = 1024, 4096, 11008
N_CORES = 8
OC = OUT // N_CORES  # 1376 output columns per core
GS = 128  # quantization group size (== matmul k-tile)
G = IN // GS  # 32 groups
PACK = 8  # int4 values per int32 word
# reference unpacks nibble k to logical column AWQ_REVERSE_ORDER.index(k);
# equivalently logical column j within a word uses shift 4*REV[j]:
REV = np.array([0, 4, 1, 5, 2, 6, 3, 7], dtype=np.uint32)

MM_N = 512  # moving-operand free size per matmul (one PSUM bank of fp32)

_CACHE = {}


def _unpack_int4(q: np.ndarray) -> np.ndarray:
    """[rows, cols//8] int32 -> [rows, cols] uint8 in 0..15 (AWQ order)."""
    qu = q.view(np.uint32)
    nib = (qu[:, :, None] >> (REV * 4)[None, None, :]) & 0xF
    return nib.reshape(q.shape[0], -1).astype(np.uint8)


def _build(m, k, oc, n_cores):
    import concourse.bacc as bacc
    import concourse.tile as tile
    from concourse import mybir

    F16 = mybir.dt.float16
    F32 = mybir.dt.float32
    U8 = mybir.dt.uint8

    g = k // GS
    n_otiles = (oc + 127) // 128
    n_mch = (m + MM_N - 1) // MM_N

    nc = bacc.Bacc("TRN2", target_bir_lowering=False, debug=False)
    xT = nc.dram_tensor("xT", [k, m], F16, kind="ExternalInput").ap()
    qw8T = nc.dram_tensor("qw8T", [oc, k], U8, kind="ExternalInput").ap()
    sT = nc.dram_tensor("sT", [oc, g], F32, kind="ExternalInput").ap()
    z8T = nc.dram_tensor("z8T", [oc, g], U8, kind="ExternalInput").ap()
    biasT = nc.dram_tensor("biasT", [oc, 1], F16, kind="ExternalInput").ap()
    outT = nc.dram_tensor("outT", [oc, m], F16, kind="ExternalOutput").ap()

    # quarter-otile granularity: groups processed in blocks of QG so the
    # first matmuls are unblocked early and otile handoff stays smooth
    QG = min(8, g)
    n_q = g // QG
    # dequant engine per group position within a quarter: 5 DVE, 3 ACT
    # (GPSIMD intentionally unused: it port-muxes with DVE and is ~7x slower)
    deq_pattern = [0, 1, 0, 0, 1, 0, 0, 1]

    with tile.TileContext(nc) as tc:
        with (
            tc.tile_pool(name="x", bufs=1) as xpool,
            tc.tile_pool(name="consts", bufs=3) as cpool,
            tc.tile_pool(name="qw", bufs=3) as qwpool,
            tc.tile_pool(name="wd", bufs=3 * n_q) as wdpool,
            tc.tile_pool(name="w", bufs=3 * n_q) as wpool,
            tc.tile_pool(name="ps", bufs=8, space="PSUM") as pspool,
            tc.tile_pool(name="o", bufs=4) as opool,
        ):
            # resident transposed activations: [128, g, m]
            xT_sb = xpool.tile([128, g, m], F16)
            xT_r = xT.rearrange("(gg p) mm -> p gg mm", p=128)

            def load_x(g0, g1):
                nc.sync.dma_start(xT_sb[:, g0:g1], xT_r[:, g0:g1])

            def prep_otile(ot):
                o0 = ot * 128
                ob = min(128, oc - o0)

                s_t = cpool.tile([ob, g], F32, tag="sT")
                z8_t = cpool.tile([ob, g], U8, tag="z8")
                zf_t = cpool.tile([ob, g], F32, tag="zf")
                zs_t = cpool.tile([ob, g], F32, tag="zs")
                nzs_t = cpool.tile([ob, g], F32, tag="nzs")
                b_t = cpool.tile([ob, 1], F16, tag="b")
                nc.sync.dma_start(s_t[:], sT[o0 : o0 + ob])
                nc.sync.dma_start(z8_t[:], z8T[o0 : o0 + ob])
                nc.sync.dma_start(b_t[:], biasT[o0 : o0 + ob])
                nc.vector.tensor_copy(zf_t[:], z8_t[:])
                nc.vector.tensor_tensor(
                    zs_t[:], zf_t[:], s_t[:], mybir.AluOpType.mult
                )
                nc.vector.tensor_scalar(
                    nzs_t[:], zs_t[:], -1.0, None, mybir.AluOpType.mult
                )

                qw_t = qwpool.tile([ob, k], U8)
                for q in range(n_q):
                    qsl = slice(q * QG * GS, (q + 1) * QG * GS)
                    nc.gpsimd.dma_start(qw_t[:, qsl], qw8T[o0 : o0 + ob, qsl])

                # dequant + transpose per quarter (QG groups each)
                w3_qs = []
                for q in range(n_q):
                    wd_t = wdpool.tile([ob, QG * GS], F16, tag="wd")
                    for j in range(QG):
                        gi = q * QG + j
                        ksl_q = slice(j * GS, (j + 1) * GS)
                        ksl = slice(gi * GS, (gi + 1) * GS)
                        if deq_pattern[j % len(deq_pattern)]:
                            nc.scalar.activation(
                                wd_t[:, ksl_q],
                                qw_t[:, ksl],
                                mybir.ActivationFunctionType.Identity,
                                bias=nzs_t[:, gi : gi + 1],
                                scale=s_t[:, gi : gi + 1],
                            )
                        else:
                            nc.vector.tensor_scalar(
                                wd_t[:, ksl_q],
                                qw_t[:, ksl],
                                s_t[:, gi : gi + 1],
                                zs_t[:, gi : gi + 1],
                                mybir.AluOpType.mult,
                                mybir.AluOpType.subtract,
                            )
                    # xbar transpose [ob, QG*GS] -> [128, QG, ob]
                    w3_t = wpool.tile([128, QG, ob], F16, tag="w")
                    nc.sync.dma_start_transpose(w3_t[:], wd_t[:])
                    w3_qs.append(w3_t)
                return o0, ob, b_t, w3_qs

            def mm_otile(ot, prep):
                # interleave the m-chunk accumulation chains per group so each
                # weight quarter is consumed in one dense burst of PE work
                o0, ob, b_t, w3_qs = prep
                mslices = [
                    slice(mc * MM_N, min(m, (mc + 1) * MM_N)) for mc in range(n_mch)
                ]
                pss = [
                    pspool.tile([ob, MM_N], F32, name=f"ps_{ot}_{mc}", tag="ps")
                    for mc in range(n_mch)
                ]
                for gi in range(g):
                    for mc in range(n_mch):
                        msl = mslices[mc]
                        nc.tensor.matmul(
                            pss[mc][:, : msl.stop - msl.start],
                            w3_qs[gi // QG][:, gi % QG, :],
                            xT_sb[:, gi, msl],
                            start=(gi == 0),
                            stop=(gi == g - 1),
                        )
                for mc in range(n_mch):
                    msl = mslices[mc]
                    mn = msl.stop - msl.start
                    o_t = opool.tile([ob, MM_N], F16)
                    nc.scalar.activation(
                        o_t[:, :mn],
                        pss[mc][:, :mn],
                        mybir.ActivationFunctionType.Identity,
                        bias=b_t[:],
                        scale=1.0,
                    )
                    nc.sync.dma_start(outT[o0 : o0 + ob, msl], o_t[:, :mn])

            prep = [prep_otile(0)]
            load_x(0, g)
            prep.append(prep_otile(1))
            for ot in range(n_otiles):
                if ot + 2 < n_otiles:
                    prep.append(prep_otile(ot + 2))
                mm_otile(ot, prep.pop(0))

    nc.compile()
    return nc


def _get_nc(m=M, k=IN, oc=OC, n_cores=N_CORES):
    key = (m, k, oc, n_cores)
    if key not in _CACHE:
        _CACHE[key] = _build(*key)
    return _CACHE[key]


def _make_in_maps(x, qweight, qzeros, scales, bias, n_cores=N_CORES):
    iw8 = _unpack_int4(qweight)  # [IN, OUT] uint8
    iz8 = _unpack_int4(qzeros)  # [G, OUT] uint8
    xT = np.ascontiguousarray(x.T)  # [IN, M] fp16
    oc = qweight.shape[1] * PACK // n_cores
    in_maps = []
    for c in range(n_cores):
        sl = slice(c * oc, (c + 1) * oc)
        in_maps.append(
            {
                "xT": xT,
                "qw8T": np.ascontiguousarray(iw8[:, sl].T),
                "sT": np.ascontiguousarray(scales[:, sl].T.astype(np.float32)),
                "z8T": np.ascontiguousarray(iz8[:, sl].T),
                "biasT": np.ascontiguousarray(bias[sl].reshape(-1, 1)),
            }
        )
    return in_maps


LAST_EXEC_NS = None


def kernel(x, qweight, qzeros, scales, bias):
    global LAST_EXEC_NS
    from concourse.bass_utils import run_bass_kernel_spmd

    x = np.asarray(x)
    qweight = np.asarray(qweight)
    qzeros = np.asarray(qzeros)
    scales = np.asarray(scales)
    bias = np.asarray(bias)

    nc = _get_nc()
    in_maps = _make_in_maps(x, qweight, qzeros, scales, bias)

    kwargs = {}
    if os.environ.get("AWQ_PROFILE"):
        _enable_profiling()
        kwargs = dict(trace=True, tmpdir=os.environ.get("AWQ_TRACE_DIR") or None)
    res = run_bass_kernel_spmd(nc, in_maps, list(range(N_CORES)), **kwargs)
    LAST_EXEC_NS = res.exec_time_ns

    outT = np.concatenate([res.results[c]["outT"] for c in range(N_CORES)], axis=0)
    return np.ascontiguousarray(outT.T)


def _enable_profiling():
    """Register the NTFF profile hook missing from this image's antenv."""
    import types

    if "antenv.axon_hooks" not in sys.modules:
        import antenv

        mod = types.ModuleType("antenv.axon_hooks")
        mod._hook = None
        mod.set_axon_ntff_profile_hook = lambda h: setattr(mod, "_hook", h)
        mod.get_axon_ntff_profile_hook = lambda: mod._hook
        sys.modules["antenv.axon_hooks"] = mod
        antenv.axon_hooks = mod
        try:
            from trn_agent_boot.trn_boot import _ntff_profile_via_ctypes

            mod.set_axon_ntff_profile_hook(
                _ntff_profile_via_ctypes("/opt/axon/libaxon_pjrt.so")
            )
        except Exception:
            pass
    import concourse.bass_utils as _bu

    _bu.upload_artifacts = lambda tmpdir: "local://skipped"
